# revision 6
# baseline (speedup 1.0000x reference)
"""Swin-window attention (B=2048, N=64, C=512, H=16) on 8 trn2 NeuronCores.

Data-parallel over windows: each core computes 256 windows end-to-end
(qkv -> per-head scores+softmax(+rel-pos bias) -> attn@v -> out proj),
returning both `out` [2048,64,512] and `attn_map` [2048,16,64,64].
"""

from contextlib import ExitStack

import numpy as np

import concourse.bass as bass
import concourse.tile as tile
from concourse import bacc, masks, mybir
from concourse.bass_utils import run_bass_kernel_spmd

F32 = mybir.dt.float32
F32R = mybir.dt.float32r
BF16 = mybir.dt.bfloat16
AF = mybir.ActivationFunctionType
AX = mybir.AxisListType

N_CORES = 8
B_FULL = 2048
W = 8          # windows per block
NT = 64        # tokens per window
C = 512
H = 16
D = 32
TOK = W * NT
N_BLOCKS = B_FULL // N_CORES // W   # 32
B_LOC = N_BLOCKS * W
T_LOC = B_LOC * NT


def build(n_blocks: int = N_BLOCKS):
    t_loc = n_blocks * TOK
    b_loc = n_blocks * W
    nc = bacc.Bacc("TRN2", target_bir_lowering=False, debug=False)

    x_d = nc.dram_tensor("x", [t_loc, C], F32, kind="ExternalInput")
    qkvw_d = nc.dram_tensor("qkvw", [C, 3 * C], F32, kind="ExternalInput")
    projw_d = nc.dram_tensor("projw", [C, C], F32, kind="ExternalInput")
    projb_d = nc.dram_tensor("projb", [128, C], F32, kind="ExternalInput")
    bias_d = nc.dram_tensor("biasarr", [8, 128, NT], F32, kind="ExternalInput")
    y_d = nc.dram_tensor("y", [t_loc, C], F32, kind="ExternalOutput")
    attn_d = nc.dram_tensor("attn", [b_loc, H, NT, NT], F32, kind="ExternalOutput")

    with tile.TileContext(nc) as tc, ExitStack() as ctx:
        const = ctx.enter_context(tc.tile_pool(name="const", bufs=1))
        stage = ctx.enter_context(tc.tile_pool(name="stage", bufs=2))
        xb_p = ctx.enter_context(tc.tile_pool(name="xb", bufs=6))
        xT_p = ctx.enter_context(tc.tile_pool(name="xT", bufs=8))
        qkT_p = ctx.enter_context(tc.tile_pool(name="qkT", bufs=12))
        v_p = ctx.enter_context(tc.tile_pool(name="v", bufs=8))
        P_p = ctx.enter_context(tc.tile_pool(name="P", bufs=3))
        P2_p = ctx.enter_context(tc.tile_pool(name="P2", bufs=3))
        A_p = ctx.enter_context(tc.tile_pool(name="A", bufs=3))
        rs_p = ctx.enter_context(tc.tile_pool(name="rs", bufs=4))
        AT_p = ctx.enter_context(tc.tile_pool(name="AT", bufs=12))
        OT_p = ctx.enter_context(tc.tile_pool(name="OT", bufs=6))
        y_p = ctx.enter_context(tc.tile_pool(name="y", bufs=4))

        sps_p = ctx.enter_context(tc.tile_pool(name="sps", bufs=2, space="PSUM"))
        big_ps = ctx.enter_context(tc.tile_pool(name="bigps", bufs=2, space="PSUM"))
        tr_ps = ctx.enter_context(tc.tile_pool(name="trps", bufs=2, space="PSUM"))
        ot_ps = ctx.enter_context(tc.tile_pool(name="otps", bufs=1, space="PSUM"))
        # head-pair j <-> heads (hA, hA+4), hA = j%4 + 8*(j//4); both share
        # PE row-group j%4 so each S psum bank is written by ONE row-group
        # (concurrent same-bank writes from different row-groups fault).

        ident = const.tile([128, 128], F32)
        masks.make_identity(nc, ident[:])

        qkvw_r = []
        for c in range(4):
            st = stage.tile([128, 3 * C], F32, tag="wstage")
            nc.sync.dma_start(st[:], qkvw_d[c * 128:(c + 1) * 128, :])
            wr = const.tile([128, 3 * C], F32R, tag=f"qkvw{c}")
            nc.any.tensor_copy(wr[:], st[:])
            qkvw_r.append(wr)
        projw_r = []
        for c in range(4):
            st = stage.tile([128, C], F32, tag="wstage2")
            nc.sync.dma_start(st[:], projw_d[c * 128:(c + 1) * 128, :])
            wr = const.tile([128, C], F32R, tag=f"projw{c}")
            nc.any.tensor_copy(wr[:], st[:])
            projw_r.append(wr)
        projb = const.tile([128, C], F32)
        nc.sync.dma_start(projb[:], projb_d[:])
        bias_sb = const.tile([128, 8, NT], F32)
        nc.sync.dma_start(bias_sb[:], bias_d[:].rearrange("j p m -> p j m"))

        for b in range(n_blocks):
            tok0 = b * TOK

            xb = []
            for t in range(4):
                xt = xb_p.tile([128, C], F32, tag="xb")
                nc.sync.dma_start(xt[:], x_d[tok0 + t * 128: tok0 + (t + 1) * 128, :])
                xb.append(xt)

            xT = []
            for c in range(4):
                tp = tr_ps.tile([128, 512], F32, tag="trps")
                for t in range(4):
                    nc.tensor.transpose(
                        tp[:, t * 128:(t + 1) * 128],
                        xb[t][:, c * 128:(c + 1) * 128],
                        ident[:],
                    )
                xc = xT_p.tile([128, TOK], F32R, tag="xT")
                nc.any.tensor_copy(xc[:], tp[:])
                xT.append(xc)

            qkT = []
            for g in range(8):
                ps = big_ps.tile([128, TOK], F32, tag="bigps")
                for c in range(4):
                    nc.tensor.matmul(
                        ps[:],
                        qkvw_r[c][:, g * 128:(g + 1) * 128],
                        xT[c][:],
                        start=(c == 0),
                        stop=(c == 3),
                    )
                qt = qkT_p.tile([128, TOK], BF16, tag="qkT")
                nc.any.tensor_copy(qt[:], ps[:])
                qkT.append(qt)

            v_sb = []
            for t in range(4):
                ps = big_ps.tile([128, C], F32, tag="bigps")
                for c in range(4):
                    nc.tensor.matmul(
                        ps[:],
                        xT[c][:, t * 128:(t + 1) * 128],
                        qkvw_r[c][:, 1024:1536],
                        start=(c == 0),
                        stop=(c == 3),
                    )
                vt = v_p.tile([128, C], BF16, tag="v")
                nc.any.tensor_copy(vt[:], ps[:])
                v_sb.append(vt)

            AT_sb = []
            for j in range(8):
                hA = j % 4 + 8 * (j // 4)
                r = 32 * (j % 4)
                sps = sps_p.tile([128, TOK], F32, tag="sps")
                for w in range(W):
                    for hp in range(2):
                        h = hA + 4 * hp
                        gq, gk = h // 4, 4 + h // 4
                        nc.tensor.matmul(
                            sps[64 * hp:64 * hp + 64, w * NT:(w + 1) * NT],
                            qkT[gq][r:r + 32, w * NT:(w + 1) * NT],
                            qkT[gk][r:r + 32, w * NT:(w + 1) * NT],
                            start=True,
                            stop=True,
                            tile_position=(r, 64 * hp),
                        )
                P = P_p.tile([128, W, NT], F32, tag="P")
                nc.vector.tensor_add(
                    P[:],
                    sps[:].rearrange("p (w m) -> p w m", w=W),
                    bias_sb[:, j:j + 1, :].broadcast_to([128, W, NT]),
                )
                P2 = P2_p.tile([128, W * NT], F32, tag="P2")
                nc.scalar.activation(P2[:], P[:].rearrange("p w m -> p (w m)"), AF.Exp)
                rs = rs_p.tile([128, W], F32, tag="rs")
                nc.vector.reduce_sum(
                    rs[:], P2[:].rearrange("p (w m) -> p w m", w=W), axis=AX.X
                )
                rinv = rs_p.tile([128, W], F32, tag="rinv")
                nc.vector.reciprocal(rinv[:], rs[:])
                A = A_p.tile([128, W * NT], F32, tag="A")
                nc.vector.tensor_mul(
                    A[:].rearrange("p (w m) -> p w m", w=W),
                    P2[:].rearrange("p (w m) -> p w m", w=W),
                    rinv[:].broadcast_to([128, W, NT]),
                )
                for hp in range(2):
                    dst = attn_d[b * W:(b + 1) * W, hA + 4 * hp, :, :].rearrange(
                        "w n m -> n w m"
                    )
                    nc.sync.dma_start(
                        dst,
                        A[64 * hp:64 * hp + 64, :].rearrange("n (w m) -> n w m", w=W),
                    )
                tp = tr_ps.tile([128, 512], F32, tag="trps")
                for wp in range(4):
                    nc.tensor.transpose(
                        tp[:, wp * 128:(wp + 1) * 128],
                        A[:, wp * 128:(wp + 1) * 128],
                        ident[:],
                    )
                at = AT_p.tile([128, 512], BF16, tag="AT")
                nc.any.tensor_copy(at[:], tp[:])
                AT_sb.append(at)

            OT_sb = []
            for cc in range(4):
                # separate psum banks per window-parity (PE row-group 0 vs 64)
                ops_e = ot_ps.tile([128, 4, NT], F32, tag="otps_e")
                ops_o = ot_ps.tile([128, 4, NT], F32, tag="otps_o")
                for w in range(W):
                    for hh in range(4):
                        h = 4 * cc + hh
                        j = h % 4 + 4 * (h // 8)
                        hpar = (h // 4) % 2
                        wp, wlo = w // 2, w % 2
                        ops = ops_o if wlo else ops_e
                        nc.tensor.matmul(
                            ops[32 * hh:32 * hh + 32, wp, :],
                            v_sb[wp][64 * wlo:64 * wlo + 64, h * D:(h + 1) * D],
                            AT_sb[j][64 * wlo:64 * wlo + 64,
                                     wp * 128 + hpar * 64: wp * 128 + hpar * 64 + 64],
                            start=True,
                            stop=True,
                            tile_position=(64 * wlo, 32 * hh),
                        )
                ot = OT_p.tile([128, TOK], F32R, tag="OT")
                otv = ot[:].rearrange("p (wp wl n) -> p wp wl n", wl=2, n=NT)
                nc.any.tensor_copy(otv[:, :, 0, :], ops_e[:])
                nc.any.tensor_copy(otv[:, :, 1, :], ops_o[:])
                OT_sb.append(ot)

            for t in range(4):
                ps = big_ps.tile([128, C], F32, tag="bigps")
                for cc in range(4):
                    nc.tensor.matmul(
                        ps[:],
                        OT_sb[cc][:, t * 128:(t + 1) * 128],
                        projw_r[cc][:],
                        start=(cc == 0),
                        stop=(cc == 3),
                    )
                yt = y_p.tile([128, C], F32, tag="y")
                nc.vector.tensor_add(yt[:], ps[:], projb[:])
                nc.sync.dma_start(
                    y_d[tok0 + t * 128: tok0 + (t + 1) * 128, :], yt[:]
                )

    nc.compile()
    return nc


def host_prep_shared(qkv_w, proj_w, proj_b, bias_table, rel_index):
    qkvw = np.array(qkv_w, np.float32, copy=True)
    qkvw[:, :C] *= np.float32(D) ** np.float32(-0.5)
    projb = np.ascontiguousarray(
        np.broadcast_to(np.asarray(proj_b, np.float32)[None, :], (128, C))
    )
    bt = np.asarray(bias_table, np.float32)
    ri = np.asarray(rel_index).astype(np.int64).reshape(-1)
    bias_hnm = bt[ri].reshape(NT, NT, H).transpose(2, 0, 1)  # [H, n, m]
    # pair j holds heads (hA, hA+4) stacked on partitions, hA = j%4 + 8*(j//4)
    biasarr = np.empty((8, 128, NT), np.float32)
    for j in range(8):
        hA = j % 4 + 8 * (j // 4)
        biasarr[j, :NT] = bias_hnm[hA]
        biasarr[j, NT:] = bias_hnm[hA + 4]
    return {
        "qkvw": np.ascontiguousarray(qkvw),
        "projw": np.ascontiguousarray(np.asarray(proj_w, np.float32)),
        "projb": projb,
        "biasarr": biasarr,
    }


_NC_CACHE = {}


def kernel(x, qkv_w, proj_w, proj_b, bias_table, rel_index):
    x = np.asarray(x, np.float32)
    B = x.shape[0]
    assert B == B_FULL and x.shape[1] == NT and x.shape[2] == C

    if N_BLOCKS not in _NC_CACHE:
        _NC_CACHE[N_BLOCKS] = build(N_BLOCKS)
    nc = _NC_CACHE[N_BLOCKS]

    shared = host_prep_shared(qkv_w, proj_w, proj_b, bias_table, rel_index)
    in_maps = []
    for i in range(N_CORES):
        xs = np.ascontiguousarray(
            x[i * B_LOC:(i + 1) * B_LOC].reshape(T_LOC, C)
        )
        in_maps.append({"x": xs, **shared})

    res = run_bass_kernel_spmd(nc, in_maps, list(range(N_CORES)))

    out = np.empty((B_FULL, NT, C), np.float32)
    attn = np.empty((B_FULL, H, NT, NT), np.float32)
    for i in range(N_CORES):
        out[i * B_LOC:(i + 1) * B_LOC] = res.results[i]["y"].reshape(B_LOC, NT, C)
        attn[i * B_LOC:(i + 1) * B_LOC] = res.results[i]["attn"]
    return out, attn


# revision 8
# speedup vs baseline: 1.0740x; 1.0740x over previous
"""Swin-window attention (B=2048, N=64, C=512, H=16) on 8 trn2 NeuronCores.

Data-parallel over windows: each core computes 256 windows end-to-end
(qkv -> per-head scores+softmax(+rel-pos bias) -> attn@v -> out proj),
returning both `out` [2048,64,512] and `attn_map` [2048,16,64,64].
"""

from contextlib import ExitStack

import numpy as np

import concourse.bass as bass
import concourse.tile as tile
from concourse import bacc, masks, mybir
from concourse.bass_utils import run_bass_kernel_spmd

F32 = mybir.dt.float32
F32R = mybir.dt.float32r
BF16 = mybir.dt.bfloat16
AF = mybir.ActivationFunctionType
AX = mybir.AxisListType

N_CORES = 8
B_FULL = 2048
W = 8          # windows per block
NT = 64        # tokens per window
C = 512
H = 16
D = 32
TOK = W * NT
N_BLOCKS = B_FULL // N_CORES // W   # 32
B_LOC = N_BLOCKS * W
T_LOC = B_LOC * NT


def build(n_blocks: int = N_BLOCKS):
    t_loc = n_blocks * TOK
    b_loc = n_blocks * W
    nc = bacc.Bacc("TRN2", target_bir_lowering=False, debug=False)

    x_d = nc.dram_tensor("x", [t_loc, C], F32, kind="ExternalInput")
    qkvw_d = nc.dram_tensor("qkvw", [C, 3 * C], F32, kind="ExternalInput")
    projw_d = nc.dram_tensor("projw", [C, C], F32, kind="ExternalInput")
    projb_d = nc.dram_tensor("projb", [128, C], F32, kind="ExternalInput")
    bias_d = nc.dram_tensor("biasarr", [8, 128, NT], F32, kind="ExternalInput")
    y_d = nc.dram_tensor("y", [t_loc, C], F32, kind="ExternalOutput")
    attn_d = nc.dram_tensor("attn", [b_loc, H, NT, NT], F32, kind="ExternalOutput")

    with tile.TileContext(nc) as tc, ExitStack() as ctx:
        const = ctx.enter_context(tc.tile_pool(name="const", bufs=1))
        stage = ctx.enter_context(tc.tile_pool(name="stage", bufs=2))
        xb_p = ctx.enter_context(tc.tile_pool(name="xb", bufs=6))
        xT_p = ctx.enter_context(tc.tile_pool(name="xT", bufs=8))
        qkT_p = ctx.enter_context(tc.tile_pool(name="qkT", bufs=12))
        v_p = ctx.enter_context(tc.tile_pool(name="v", bufs=8))
        P_p = ctx.enter_context(tc.tile_pool(name="P", bufs=3))
        P2_p = ctx.enter_context(tc.tile_pool(name="P2", bufs=3))
        A_p = ctx.enter_context(tc.tile_pool(name="A", bufs=10))
        rs_p = ctx.enter_context(tc.tile_pool(name="rs", bufs=4))
        AT_p = ctx.enter_context(tc.tile_pool(name="AT", bufs=12))
        OT_p = ctx.enter_context(tc.tile_pool(name="OT", bufs=6))
        y_p = ctx.enter_context(tc.tile_pool(name="y", bufs=4))

        sps_p = ctx.enter_context(tc.tile_pool(name="sps", bufs=2, space="PSUM"))
        big_ps = ctx.enter_context(tc.tile_pool(name="bigps", bufs=2, space="PSUM"))
        tr_ps = ctx.enter_context(tc.tile_pool(name="trps", bufs=2, space="PSUM"))
        ot_ps = ctx.enter_context(tc.tile_pool(name="otps", bufs=1, space="PSUM"))
        # head-pair j <-> heads (hA, hA+4), hA = j%4 + 8*(j//4); both share
        # PE row-group j%4 so each S psum bank is written by ONE row-group
        # (concurrent same-bank writes from different row-groups fault).

        ident = const.tile([128, 128], F32)
        masks.make_identity(nc, ident[:])

        qkvw_r = []
        for c in range(4):
            st = stage.tile([128, 3 * C], F32, tag="wstage")
            nc.sync.dma_start(st[:], qkvw_d[c * 128:(c + 1) * 128, :])
            wr = const.tile([128, 3 * C], F32R, tag=f"qkvw{c}")
            nc.any.tensor_copy(wr[:], st[:])
            qkvw_r.append(wr)
        projw_r = []
        for c in range(4):
            st = stage.tile([128, C], F32, tag="wstage2")
            nc.sync.dma_start(st[:], projw_d[c * 128:(c + 1) * 128, :])
            wr = const.tile([128, C], F32R, tag=f"projw{c}")
            nc.any.tensor_copy(wr[:], st[:])
            projw_r.append(wr)
        projb = const.tile([128, C], F32)
        nc.sync.dma_start(projb[:], projb_d[:])
        bias_sb = const.tile([128, 8, NT], F32)
        nc.sync.dma_start(bias_sb[:], bias_d[:].rearrange("j p m -> p j m"))

        for b in range(n_blocks):
            tok0 = b * TOK

            xb = []
            for t in range(4):
                xt = xb_p.tile([128, C], F32, tag="xb")
                nc.sync.dma_start(xt[:], x_d[tok0 + t * 128: tok0 + (t + 1) * 128, :])
                xb.append(xt)

            xT = []
            for c in range(4):
                tp = tr_ps.tile([128, 512], F32, tag="trps")
                for t in range(4):
                    nc.tensor.transpose(
                        tp[:, t * 128:(t + 1) * 128],
                        xb[t][:, c * 128:(c + 1) * 128],
                        ident[:],
                    )
                xc = xT_p.tile([128, TOK], F32R, tag="xT")
                nc.vector.tensor_copy(xc[:], tp[:])
                xT.append(xc)

            qkT = []
            for g in range(8):
                ps = big_ps.tile([128, TOK], F32, tag="bigps")
                for c in range(4):
                    nc.tensor.matmul(
                        ps[:],
                        qkvw_r[c][:, g * 128:(g + 1) * 128],
                        xT[c][:],
                        start=(c == 0),
                        stop=(c == 3),
                    )
                qt = qkT_p.tile([128, TOK], BF16, tag="qkT")
                if g % 2 == 0:
                    nc.vector.tensor_copy(qt[:], ps[:])
                else:
                    nc.scalar.copy(qt[:], ps[:])
                qkT.append(qt)

            v_sb = []
            for t in range(4):
                ps = big_ps.tile([128, C], F32, tag="bigps")
                for c in range(4):
                    nc.tensor.matmul(
                        ps[:],
                        xT[c][:, t * 128:(t + 1) * 128],
                        qkvw_r[c][:, 1024:1536],
                        start=(c == 0),
                        stop=(c == 3),
                    )
                vt = v_p.tile([128, C], BF16, tag="v")
                nc.scalar.copy(vt[:], ps[:])
                v_sb.append(vt)

            AT_sb = []
            A_tiles = []
            for j in range(8):
                hA = j % 4 + 8 * (j // 4)
                r = 32 * (j % 4)
                sps = sps_p.tile([128, TOK], F32, tag="sps")
                for w in range(W):
                    for hp in range(2):
                        h = hA + 4 * hp
                        gq, gk = h // 4, 4 + h // 4
                        nc.tensor.matmul(
                            sps[64 * hp:64 * hp + 64, w * NT:(w + 1) * NT],
                            qkT[gq][r:r + 32, w * NT:(w + 1) * NT],
                            qkT[gk][r:r + 32, w * NT:(w + 1) * NT],
                            start=True,
                            stop=True,
                            tile_position=(r, 64 * hp),
                        )
                P = P_p.tile([128, W, NT], F32, tag="P")
                nc.vector.tensor_add(
                    P[:],
                    sps[:].rearrange("p (w m) -> p w m", w=W),
                    bias_sb[:, j:j + 1, :].broadcast_to([128, W, NT]),
                )
                P2 = P2_p.tile([128, W * NT], F32, tag="P2")
                nc.scalar.activation(P2[:], P[:].rearrange("p w m -> p (w m)"), AF.Exp)
                rs = rs_p.tile([128, W], F32, tag="rs")
                nc.vector.reduce_sum(
                    rs[:], P2[:].rearrange("p (w m) -> p w m", w=W), axis=AX.X
                )
                rinv = rs_p.tile([128, W], F32, tag="rinv")
                nc.vector.reciprocal(rinv[:], rs[:])
                A = A_p.tile([128, W * NT], F32, tag="A")
                nc.gpsimd.tensor_mul(
                    A[:].rearrange("p (w m) -> p w m", w=W),
                    P2[:].rearrange("p (w m) -> p w m", w=W),
                    rinv[:].broadcast_to([128, W, NT]),
                )
                for hp in range(2):
                    dst = attn_d[b * W:(b + 1) * W, hA + 4 * hp, :, :].rearrange(
                        "w n m -> n w m"
                    )
                    nc.sync.dma_start(
                        dst,
                        A[64 * hp:64 * hp + 64, :].rearrange("n (w m) -> n w m", w=W),
                    )
                A_tiles.append(A)
            for j in range(8):
                A = A_tiles[j]
                tp = tr_ps.tile([128, 512], F32, tag="trps")
                for wp in range(4):
                    nc.tensor.transpose(
                        tp[:, wp * 128:(wp + 1) * 128],
                        A[:, wp * 128:(wp + 1) * 128],
                        ident[:],
                    )
                at = AT_p.tile([128, 512], BF16, tag="AT")
                if j % 2 == 0:
                    nc.vector.tensor_copy(at[:], tp[:])
                else:
                    nc.scalar.copy(at[:], tp[:])
                AT_sb.append(at)

            OT_sb = []
            for cc in range(4):
                # separate psum banks per window-parity (PE row-group 0 vs 64)
                ops_e = ot_ps.tile([128, 4, NT], F32, tag="otps_e")
                ops_o = ot_ps.tile([128, 4, NT], F32, tag="otps_o")
                for w in range(W):
                    for hh in range(4):
                        h = 4 * cc + hh
                        j = h % 4 + 4 * (h // 8)
                        hpar = (h // 4) % 2
                        wp, wlo = w // 2, w % 2
                        ops = ops_o if wlo else ops_e
                        nc.tensor.matmul(
                            ops[32 * hh:32 * hh + 32, wp, :],
                            v_sb[wp][64 * wlo:64 * wlo + 64, h * D:(h + 1) * D],
                            AT_sb[j][64 * wlo:64 * wlo + 64,
                                     wp * 128 + hpar * 64: wp * 128 + hpar * 64 + 64],
                            start=True,
                            stop=True,
                            tile_position=(64 * wlo, 32 * hh),
                        )
                ot = OT_p.tile([128, TOK], F32R, tag="OT")
                otv = ot[:].rearrange("p (wp wl n) -> p wp wl n", wl=2, n=NT)
                nc.vector.tensor_copy(otv[:, :, 0, :], ops_e[:])
                nc.scalar.copy(otv[:, :, 1, :], ops_o[:])
                OT_sb.append(ot)

            for t in range(4):
                ps = big_ps.tile([128, C], F32, tag="bigps")
                for cc in range(4):
                    nc.tensor.matmul(
                        ps[:],
                        OT_sb[cc][:, t * 128:(t + 1) * 128],
                        projw_r[cc][:],
                        start=(cc == 0),
                        stop=(cc == 3),
                    )
                yt = y_p.tile([128, C], F32, tag="y")
                nc.vector.tensor_add(yt[:], ps[:], projb[:])
                nc.sync.dma_start(
                    y_d[tok0 + t * 128: tok0 + (t + 1) * 128, :], yt[:]
                )

    nc.compile()
    return nc


def host_prep_shared(qkv_w, proj_w, proj_b, bias_table, rel_index):
    qkvw = np.array(qkv_w, np.float32, copy=True)
    qkvw[:, :C] *= np.float32(D) ** np.float32(-0.5)
    projb = np.ascontiguousarray(
        np.broadcast_to(np.asarray(proj_b, np.float32)[None, :], (128, C))
    )
    bt = np.asarray(bias_table, np.float32)
    ri = np.asarray(rel_index).astype(np.int64).reshape(-1)
    bias_hnm = bt[ri].reshape(NT, NT, H).transpose(2, 0, 1)  # [H, n, m]
    # pair j holds heads (hA, hA+4) stacked on partitions, hA = j%4 + 8*(j//4)
    biasarr = np.empty((8, 128, NT), np.float32)
    for j in range(8):
        hA = j % 4 + 8 * (j // 4)
        biasarr[j, :NT] = bias_hnm[hA]
        biasarr[j, NT:] = bias_hnm[hA + 4]
    return {
        "qkvw": np.ascontiguousarray(qkvw),
        "projw": np.ascontiguousarray(np.asarray(proj_w, np.float32)),
        "projb": projb,
        "biasarr": biasarr,
    }


_NC_CACHE = {}


def kernel(x, qkv_w, proj_w, proj_b, bias_table, rel_index):
    x = np.asarray(x, np.float32)
    B = x.shape[0]
    assert B == B_FULL and x.shape[1] == NT and x.shape[2] == C

    if N_BLOCKS not in _NC_CACHE:
        _NC_CACHE[N_BLOCKS] = build(N_BLOCKS)
    nc = _NC_CACHE[N_BLOCKS]

    shared = host_prep_shared(qkv_w, proj_w, proj_b, bias_table, rel_index)
    in_maps = []
    for i in range(N_CORES):
        xs = np.ascontiguousarray(
            x[i * B_LOC:(i + 1) * B_LOC].reshape(T_LOC, C)
        )
        in_maps.append({"x": xs, **shared})

    res = run_bass_kernel_spmd(nc, in_maps, list(range(N_CORES)))

    out = np.empty((B_FULL, NT, C), np.float32)
    attn = np.empty((B_FULL, H, NT, NT), np.float32)
    for i in range(N_CORES):
        out[i * B_LOC:(i + 1) * B_LOC] = res.results[i]["y"].reshape(B_LOC, NT, C)
        attn[i * B_LOC:(i + 1) * B_LOC] = res.results[i]["attn"]
    return out, attn


# revision 11
# speedup vs baseline: 1.0924x; 1.0172x over previous
"""Swin-window attention (B=2048, N=64, C=512, H=16) on 8 trn2 NeuronCores.

Data-parallel over windows: each core computes 256 windows end-to-end
(qkv -> per-head scores+softmax(+rel-pos bias) -> attn@v -> out proj),
returning both `out` [2048,64,512] and `attn_map` [2048,16,64,64].
"""

from contextlib import ExitStack

import numpy as np

import concourse.bass as bass
import concourse.tile as tile
from concourse import bacc, masks, mybir
from concourse.bass_utils import run_bass_kernel_spmd

F32 = mybir.dt.float32
F32R = mybir.dt.float32r
BF16 = mybir.dt.bfloat16
AF = mybir.ActivationFunctionType
AX = mybir.AxisListType

N_CORES = 8
B_FULL = 2048
W = 8          # windows per block
NT = 64        # tokens per window
C = 512
H = 16
D = 32
TOK = W * NT
N_BLOCKS = B_FULL // N_CORES // W   # 32
B_LOC = N_BLOCKS * W
T_LOC = B_LOC * NT


def build(n_blocks: int = N_BLOCKS):
    t_loc = n_blocks * TOK
    b_loc = n_blocks * W
    nc = bacc.Bacc("TRN2", target_bir_lowering=False, debug=False)

    x_d = nc.dram_tensor("x", [t_loc, C], F32, kind="ExternalInput")
    qkvw_d = nc.dram_tensor("qkvw", [C, 3 * C], F32, kind="ExternalInput")
    projw_d = nc.dram_tensor("projw", [C, C], F32, kind="ExternalInput")
    bias_d = nc.dram_tensor("biasarr", [8, 128, NT], F32, kind="ExternalInput")
    y_d = nc.dram_tensor("y", [t_loc, C], F32, kind="ExternalOutput")
    attn_d = nc.dram_tensor("attn", [b_loc, H, NT, NT], F32, kind="ExternalOutput")

    with tile.TileContext(nc) as tc, ExitStack() as ctx:
        const = ctx.enter_context(tc.tile_pool(name="const", bufs=1))
        stage = ctx.enter_context(tc.tile_pool(name="stage", bufs=2))
        xb_p = ctx.enter_context(tc.tile_pool(name="xb", bufs=6))
        xT_p = ctx.enter_context(tc.tile_pool(name="xT", bufs=8))
        qkT_p = ctx.enter_context(tc.tile_pool(name="qkT", bufs=12))
        v_p = ctx.enter_context(tc.tile_pool(name="v", bufs=8))
        P_p = ctx.enter_context(tc.tile_pool(name="P", bufs=3))
        P2_p = ctx.enter_context(tc.tile_pool(name="P2", bufs=3))
        A_p = ctx.enter_context(tc.tile_pool(name="A", bufs=10))
        rs_p = ctx.enter_context(tc.tile_pool(name="rs", bufs=4))
        AT_p = ctx.enter_context(tc.tile_pool(name="AT", bufs=12))
        OT_p = ctx.enter_context(tc.tile_pool(name="OT", bufs=6))
        y_p = ctx.enter_context(tc.tile_pool(name="y", bufs=4))

        sps_p = ctx.enter_context(tc.tile_pool(name="sps", bufs=2, space="PSUM"))
        big_ps = ctx.enter_context(tc.tile_pool(name="bigps", bufs=2, space="PSUM"))
        tr_ps = ctx.enter_context(tc.tile_pool(name="trps", bufs=2, space="PSUM"))
        ot_ps = ctx.enter_context(tc.tile_pool(name="otps", bufs=1, space="PSUM"))
        # head-pair j <-> heads (hA, hA+4), hA = j%4 + 8*(j//4); both share
        # PE row-group j%4 so each S psum bank is written by ONE row-group
        # (concurrent same-bank writes from different row-groups fault).

        ident = const.tile([128, 128], F32)
        masks.make_identity(nc, ident[:])

        qkvw_r = []
        for c in range(4):
            st = stage.tile([128, 3 * C], F32, tag="wstage")
            nc.sync.dma_start(st[:], qkvw_d[c * 128:(c + 1) * 128, :])
            wr = const.tile([128, 3 * C], F32R, tag=f"qkvw{c}")
            nc.any.tensor_copy(wr[:], st[:])
            qkvw_r.append(wr)
        projw_r = []
        for c in range(4):
            st = stage.tile([128, C], F32, tag="wstage2")
            nc.sync.dma_start(st[:], projw_d[c * 128:(c + 1) * 128, :])
            wr = const.tile([128, C], F32R, tag=f"projw{c}")
            nc.any.tensor_copy(wr[:], st[:])
            projw_r.append(wr)
        bias_sb = const.tile([128, 8, NT], F32)
        nc.sync.dma_start(bias_sb[:], bias_d[:].rearrange("j p m -> p j m"))

        for b in range(n_blocks):
            tok0 = b * TOK

            xb = []
            for t in range(4):
                xt = xb_p.tile([128, C], F32, tag="xb")
                nc.sync.dma_start(xt[:], x_d[tok0 + t * 128: tok0 + (t + 1) * 128, :])
                xb.append(xt)

            xT = []
            for c in range(4):
                tp = tr_ps.tile([128, 512], F32, tag="trps")
                for t in range(4):
                    nc.tensor.transpose(
                        tp[:, t * 128:(t + 1) * 128],
                        xb[t][:, c * 128:(c + 1) * 128],
                        ident[:],
                    )
                xc = xT_p.tile([128, TOK], F32R, tag="xT")
                nc.vector.tensor_copy(xc[:], tp[:])
                xT.append(xc)

            qkT = []
            for g in range(8):
                ps = big_ps.tile([128, TOK], F32, tag="bigps")
                for c in range(4):
                    nc.tensor.matmul(
                        ps[:],
                        qkvw_r[c][:, g * 128:(g + 1) * 128],
                        xT[c][:],
                        start=(c == 0),
                        stop=(c == 3),
                    )
                qt = qkT_p.tile([128, TOK], BF16, tag="qkT")
                if g % 2 == 0:
                    nc.vector.tensor_copy(qt[:], ps[:])
                else:
                    nc.scalar.copy(qt[:], ps[:])
                qkT.append(qt)

            v_sb = []
            for t in range(4):
                ps = big_ps.tile([128, C], F32, tag="bigps")
                for c in range(4):
                    nc.tensor.matmul(
                        ps[:],
                        xT[c][:, t * 128:(t + 1) * 128],
                        qkvw_r[c][:, 1024:1536],
                        start=(c == 0),
                        stop=(c == 3),
                    )
                vt = v_p.tile([128, C], BF16, tag="v")
                nc.scalar.copy(vt[:], ps[:])
                v_sb.append(vt)

            AT_sb = []
            A_tiles = []
            for j in range(8):
                hA = j % 4 + 8 * (j // 4)
                r = 32 * (j % 4)
                sps = sps_p.tile([128, TOK], F32, tag="sps")
                for w in range(W):
                    for hp in range(2):
                        h = hA + 4 * hp
                        gq, gk = h // 4, 4 + h // 4
                        nc.tensor.matmul(
                            sps[64 * hp:64 * hp + 64, w * NT:(w + 1) * NT],
                            qkT[gq][r:r + 32, w * NT:(w + 1) * NT],
                            qkT[gk][r:r + 32, w * NT:(w + 1) * NT],
                            start=True,
                            stop=True,
                            tile_position=(r, 64 * hp),
                        )
                P = P_p.tile([128, W, NT], F32, tag="P")
                nc.vector.tensor_add(
                    P[:],
                    sps[:].rearrange("p (w m) -> p w m", w=W),
                    bias_sb[:, j:j + 1, :].broadcast_to([128, W, NT]),
                )
                P2 = P2_p.tile([128, W * NT], F32, tag="P2")
                nc.scalar.activation(P2[:], P[:].rearrange("p w m -> p (w m)"), AF.Exp)
                rs = rs_p.tile([128, W], F32, tag="rs")
                nc.vector.reduce_sum(
                    rs[:], P2[:].rearrange("p (w m) -> p w m", w=W), axis=AX.X
                )
                rinv = rs_p.tile([128, W], F32, tag="rinv")
                nc.vector.reciprocal(rinv[:], rs[:])
                A = A_p.tile([128, W * NT], F32, tag="A")
                nc.gpsimd.tensor_mul(
                    A[:].rearrange("p (w m) -> p w m", w=W),
                    P2[:].rearrange("p (w m) -> p w m", w=W),
                    rinv[:].broadcast_to([128, W, NT]),
                )
                for hp in range(2):
                    dst = attn_d[b * W:(b + 1) * W, hA + 4 * hp, :, :].rearrange(
                        "w n m -> n w m"
                    )
                    nc.sync.dma_start(
                        dst,
                        A[64 * hp:64 * hp + 64, :].rearrange("n (w m) -> n w m", w=W),
                    )
                A_tiles.append(A)
            for j in range(8):
                A = A_tiles[j]
                tp = tr_ps.tile([128, 512], F32, tag="trps")
                for wp in range(4):
                    nc.tensor.transpose(
                        tp[:, wp * 128:(wp + 1) * 128],
                        A[:, wp * 128:(wp + 1) * 128],
                        ident[:],
                    )
                at = AT_p.tile([128, 512], BF16, tag="AT")
                if j % 2 == 0:
                    nc.vector.tensor_copy(at[:], tp[:])
                else:
                    nc.scalar.copy(at[:], tp[:])
                AT_sb.append(at)

            OT_sb = []
            for cc in range(4):
                # separate psum banks per window-parity (PE row-group 0 vs 64)
                ops_e = ot_ps.tile([128, 4, NT], F32, tag="otps_e")
                ops_o = ot_ps.tile([128, 4, NT], F32, tag="otps_o")
                for w in range(W):
                    for hh in range(4):
                        h = 4 * cc + hh
                        j = h % 4 + 4 * (h // 8)
                        hpar = (h // 4) % 2
                        wp, wlo = w // 2, w % 2
                        ops = ops_o if wlo else ops_e
                        nc.tensor.matmul(
                            ops[32 * hh:32 * hh + 32, wp, :],
                            v_sb[wp][64 * wlo:64 * wlo + 64, h * D:(h + 1) * D],
                            AT_sb[j][64 * wlo:64 * wlo + 64,
                                     wp * 128 + hpar * 64: wp * 128 + hpar * 64 + 64],
                            start=True,
                            stop=True,
                            tile_position=(64 * wlo, 32 * hh),
                        )
                ot = OT_p.tile([128, TOK], F32R, tag="OT")
                otv = ot[:].rearrange("p (wp wl n) -> p wp wl n", wl=2, n=NT)
                nc.vector.tensor_copy(otv[:, :, 0, :], ops_e[:])
                nc.scalar.copy(otv[:, :, 1, :], ops_o[:])
                OT_sb.append(ot)

            for t in range(4):
                ps = big_ps.tile([128, C], F32, tag="bigps")
                for cc in range(4):
                    nc.tensor.matmul(
                        ps[:],
                        OT_sb[cc][:, t * 128:(t + 1) * 128],
                        projw_r[cc][:],
                        start=(cc == 0),
                        stop=(cc == 3),
                    )
                yt = y_p.tile([128, C], F32, tag="y")
                if t % 2 == 0:
                    nc.vector.tensor_copy(yt[:], ps[:])
                else:
                    nc.scalar.copy(yt[:], ps[:])
                nc.sync.dma_start(
                    y_d[tok0 + t * 128: tok0 + (t + 1) * 128, :], yt[:]
                )

    nc.compile()
    return nc


def host_prep_shared(qkv_w, proj_w, proj_b, bias_table, rel_index):
    qkvw = np.array(qkv_w, np.float32, copy=True)
    qkvw[:, :C] *= np.float32(D) ** np.float32(-0.5)
    bt = np.asarray(bias_table, np.float32)
    ri = np.asarray(rel_index).astype(np.int64).reshape(-1)
    bias_hnm = bt[ri].reshape(NT, NT, H).transpose(2, 0, 1)  # [H, n, m]
    # pair j holds heads (hA, hA+4) stacked on partitions, hA = j%4 + 8*(j//4)
    biasarr = np.empty((8, 128, NT), np.float32)
    for j in range(8):
        hA = j % 4 + 8 * (j // 4)
        biasarr[j, :NT] = bias_hnm[hA]
        biasarr[j, NT:] = bias_hnm[hA + 4]
    return {
        "qkvw": np.ascontiguousarray(qkvw),
        "projw": np.ascontiguousarray(np.asarray(proj_w, np.float32)),
        "biasarr": biasarr,
    }


_NC_CACHE = {}


def kernel(x, qkv_w, proj_w, proj_b, bias_table, rel_index):
    x = np.asarray(x, np.float32)
    B = x.shape[0]
    assert B == B_FULL and x.shape[1] == NT and x.shape[2] == C

    if N_BLOCKS not in _NC_CACHE:
        _NC_CACHE[N_BLOCKS] = build(N_BLOCKS)
    nc = _NC_CACHE[N_BLOCKS]

    shared = host_prep_shared(qkv_w, proj_w, proj_b, bias_table, rel_index)
    in_maps = []
    for i in range(N_CORES):
        xs = np.ascontiguousarray(
            x[i * B_LOC:(i + 1) * B_LOC].reshape(T_LOC, C)
        )
        in_maps.append({"x": xs, **shared})

    res = run_bass_kernel_spmd(nc, in_maps, list(range(N_CORES)))

    out = np.empty((B_FULL, NT, C), np.float32)
    attn = np.empty((B_FULL, H, NT, NT), np.float32)
    for i in range(N_CORES):
        out[i * B_LOC:(i + 1) * B_LOC] = res.results[i]["y"].reshape(B_LOC, NT, C)
        attn[i * B_LOC:(i + 1) * B_LOC] = res.results[i]["attn"]
    pb = np.asarray(proj_b, np.float32)
    if np.any(pb):
        out += pb[None, None, :]
    return out, attn
